# revision 18
# baseline (speedup 1.0000x reference)
"""Continual-attention Trainium2 kernel (8 NeuronCores, SPMD).

Sharding: core c -> batch b = c//2, head-group g = c%2 (4 heads each).
Per (b,h) computes S^T[k,q] = K Q^T via PE, exp on ScalarE with fused
1/sqrt(d) scale, 0/1 mask multiplies on DVE, then O^T[d,q] (+ softmax
denominator as a 65th row via a ones column in V) accumulated on PE.

The test-query block (last 512 q) is permuted so its 64 chunks are
sorted by attach_test_after descending; the train-key work for it then
forms a staircase (k-tile kp only needed by the prefix of chunks with
attach >= 128*kp).  The staircase (union over the 4 batches) is baked
into the compiled program -- programs are JIT-specialized per
attach pattern and cached.  Per-core 0/1 boundary masks handle the
sub-tile thresholds (and batches whose own staircase is shorter than
the union).

S tiles are bank-packed first-fit-decreasing into [128,1536] PSUM
tiles (3 banks) so each ScalarE exp covers up to 1536 cols -- ACT is
the bottleneck engine, so fewer/larger exps with no packing holes.
Output is DMA'd straight from PSUM; normalization + transpose on host.
"""

import sys

sys.path.insert(0, "/opt/trn_rl_repo")

import numpy as np

B, L, H, D = 4, 2048, 8, 64
TRAIN = 1536
TEST = L - TRAIN            # 512
NCH = 64                    # test chunks
CH = TEST // NCH            # 8
HPC = 4                     # heads per core
NCORES = 8
KT = L // 128               # 16 k-tiles
NTR = TRAIN // 128          # 12 train k-tiles

LAST_RESULT = None          # BassKernelResults of the most recent run
_PROG_CACHE = {}            # staircase signature -> compiled Bass program


def _split_multi_waits(nc, mybir):
    """This container's walrus accepts at most one semaphore wait per
    instruction; Tile's tail drains can carry several. Hoist extras onto
    NoOps inserted immediately before, on the same engine."""
    for f in nc.m.functions:
        for bb in f.blocks:
            insts = list(bb.instructions)
            out = []
            changed = False
            for inst in insts:
                si = inst.sync_info
                if si is not None and len(si.on_wait) > 1:
                    waits = list(si.on_wait)
                    for w in waits[:-1]:
                        nop = mybir.InstNoOp(
                            name=f"waitnop-{nc.next_id()}", ins=[], outs=[]
                        )
                        nop.engine = inst.engine
                        nop.sync_info = mybir.SyncInfo(on_wait=[w], on_update=[])
                        out.append(nop)
                    inst.sync_info = mybir.SyncInfo(
                        on_wait=[waits[-1]], on_update=list(si.on_update)
                    )
                    changed = True
                out.append(inst)
            if changed:
                bb.instructions = out


def _staircase(attach):
    """From full attach [B, NCH] -> (n, lo) per train k-tile kp:
    n[kp]  = union (max over batches) #chunks needing tile kp (desc-sorted)
    lo[kp] = min over batches of #chunks fully covered by tile kp
    Both in chunk units; the kernel computes cols [0, 8*n) for tile kp and
    mask-multiplies cols [8*lo, 8*n)."""
    srt = -np.sort(-attach, axis=1)          # [B, NCH] descending
    n = np.zeros(NTR, np.int64)
    lo = np.zeros(NTR, np.int64)
    for kp in range(NTR):
        need = (srt >= 128 * kp).sum(axis=1)       # per-batch chunks needing kp
        full = (srt >= 128 * kp + 127).sum(axis=1)  # per-batch fully-allowed
        n[kp] = need.max()
        lo[kp] = full.min()
    return n, lo


def _pack_items(items):
    """items: list of (kp, off, w, kind). First item must stay first
    (its AV carries start=True over the full 512).  Pack into groups of
    <=3 PSUM banks (1536 cols), each item within a single 512-col bank,
    first-fit-decreasing after the first item.  Returns list of groups;
    each group is (span, [(kp, off, w, kind, pos), ...])."""
    first, rest = items[0], sorted(items[1:], key=lambda it: -it[2])
    banks = [[first]]  # each bank: list of items, sum w <= 512
    for it in rest:
        for bk in banks:
            if sum(x[2] for x in bk) + it[2] <= 512:
                bk.append(it)
                break
        else:
            banks.append([it])
    # full banks first (stable: kp0's full bank stays first) so partial
    # banks cluster at the end -- gaps between banks stay small
    banks.sort(key=lambda bk: -sum(x[2] for x in bk))
    groups = []
    for i in range(0, len(banks), 3):
        chunk = banks[i : i + 3]
        placed = []
        gaps = []  # unwritten [start, end) holes, zero-filled pre-exp
        pos = 0
        for j, bk in enumerate(chunk):
            if j and pos < 512 * j:
                gaps.append((pos, 512 * j))
            pos = 512 * j
            for it in bk:
                placed.append((*it, pos))
                pos += it[2]
        groups.append((pos, gaps, placed))
    return groups


def _gq_items(gq, n):
    """Item list for query-group gq. kind: 0 plain, 1 diag, 2 chunk-block,
    3 staircase (mask cols [8*lo,8*n))."""
    items = []
    if gq < 3:
        for kp in range(4 * (gq + 1)):
            off = max(0, 128 * kp - 512 * gq)
            kind = 1 if 128 * kp >= 512 * gq else 0
            items.append((kp, off, 512 - off, kind))
    else:
        for kp in range(NTR):
            if n[kp] > 0:
                items.append((kp, 0, int(8 * n[kp]), 3))
        for kp in range(NTR, KT):
            items.append((kp, 128 * (kp - NTR), 128, 2))
    return items


def _build_program(n, lo):
    import concourse.bass as bass
    import concourse.mybir as mybir
    import concourse.tile as tile

    f32 = mybir.dt.float32
    fp16 = mybir.dt.float16
    Exp = mybir.ActivationFunctionType.Exp

    band_w = [int(8 * (n[kp] - lo[kp])) for kp in range(NTR)]
    band_off = np.concatenate([[0], np.cumsum(band_w)]).astype(int)
    BAND = int(band_off[-1])

    nc = bass.Bass()

    kq_d = nc.dram_tensor("kq", [HPC, 128, 2 * L], fp16, kind="ExternalInput")
    vw_d = nc.dram_tensor("vw", [HPC, 128, KT * 65], fp16, kind="ExternalInput")
    # masks: [0:128) mdiag | [128:256) mchunk | [256:256+BAND) staircase bands
    msk_d = nc.dram_tensor("msk", [128, 256 + BAND], fp16, kind="ExternalInput")
    ot_d = nc.dram_tensor("ot", [HPC, 65, L], f32, kind="ExternalOutput")

    with tile.TileContext(nc) as tc:
        with (
            tc.tile_pool(name="consts", bufs=1) as consts,
            tc.tile_pool(name="heads", bufs=2) as heads,
            tc.tile_pool(name="ptp", bufs=5) as ptp,
            tc.tile_pool(name="osbp", bufs=3) as osbp,
            tc.tile_pool(name="spp", bufs=2, space="PSUM") as spp,
            tc.tile_pool(name="avp", bufs=2, space="PSUM") as avp,
        ):
            msk_sb = consts.tile([128, 256 + BAND], fp16)
            zt = consts.tile([128, 512], fp16)
            nc.vector.memset(zt, 0.0)
            # dummy activation: hoists the Exp table load to t~0, off the
            # critical path (otherwise it serializes before the first exp)
            warm = consts.tile([1, 8], f32)
            nc.scalar.activation(warm, zt[0:1, 0:8], Exp)

            # AV matmuls are held back PIPE groups and flow across gq/head
            # boundaries so the PE never serializes a drain in front of the
            # next group's S matmuls (which would starve ScalarE).
            PIPE = 2
            pending = []  # entries: (av, vw_sb, [(kp,pt,pos,off,w,start,stop)], fin)

            def pop_emit():
                av_e, vw_e, args, fin = pending.pop(0)
                for kp, pt, pos, off, w, st, sp in args:
                    nc.tensor.matmul(
                        av_e[:65, off : off + w],
                        lhsT=vw_e[:, kp, :],
                        rhs=pt[:, pos : pos + w],
                        start=st,
                        stop=sp,
                        skip_group_check=True,
                    )
                if fin is not None:
                    h_e, gq_e = fin
                    osb = osbp.tile([65, 512], f32)
                    nc.vector.tensor_copy(osb, av_e[:65, :])
                    nc.sync.dma_start(
                        out=ot_d.ap()[h_e][:, 512 * gq_e : 512 * gq_e + 512],
                        in_=osb,
                    )

            first = True
            for h in range(HPC):
                # qt/kt are zero-padded to 128 contraction rows: K=64 matmuls
                # never leave the PE's throttled clock state (HW-measured
                # 430ns vs 216ns per 512-col matmul), K=128 ones do.
                # head-0 DMAs are ordered/split so compute starts as soon as
                # the first k/q columns land; qt goes via the ACT-engine DGE
                # so the sync sequencer isn't the serial bottleneck.
                kq_sb = heads.tile([128, 2 * L], fp16, tag="kq")
                kt_sb = kq_sb[:, 0:L]
                qt_sb = kq_sb[:, L : 2 * L]
                vw_sb = heads.tile([128, KT, 65], fp16, tag="vw")
                kq2 = kq_sb.rearrange("p (b c) -> p b c", b=2)
                kqd2 = kq_d.ap()[h].rearrange("p (b c) -> p b c", b=2)
                if first:
                    # first chunk: both k and q cols [0:512), real rows only
                    # (fewest descriptors + one issue/sem chain), zero rows
                    # via idle gpsimd; bulk bytes issued after so they don't
                    # queue in front on the DMA engines
                    nc.gpsimd.memset(kt_sb[64:128, 0:512], 0.0)
                    nc.gpsimd.memset(qt_sb[64:128, 0:512], 0.0)
                    nc.sync.dma_start(
                        out=kq2[0:64, :, 0:512], in_=kqd2[0:64, :, 0:512]
                    )
                    nc.sync.dma_start(out=msk_sb, in_=msk_d.ap())
                    nc.scalar.dma_start(
                        out=kq2[:, :, 512:L], in_=kqd2[:, :, 512:L]
                    )
                    vw_r = vw_d.ap()[h].rearrange("p (t c) -> p t c", t=KT)
                    nc.sync.dma_start(out=vw_sb[:, 0:4, :], in_=vw_r[:, 0:4, :])
                    nc.sync.dma_start(out=vw_sb[:, 4:KT, :], in_=vw_r[:, 4:KT, :])
                    first = False
                else:
                    # on the sync DGE: a scalar-engine dma_start mid-stream
                    # would stall the exp queue for its ~630ns HWDGE issue
                    nc.sync.dma_start(out=kq_sb, in_=kq_d.ap()[h])
                    nc.sync.dma_start(
                        out=vw_sb,
                        in_=vw_d.ap()[h].rearrange("p (t c) -> p t c", t=KT),
                    )

                for gq in range(4):
                    av = avp.tile([128, 512], f32, tag="av")
                    items = _gq_items(gq, n)
                    groups = _pack_items(items)
                    first_kp = items[0][0]          # first AV emitted: start
                    last_kp = groups[-1][2][-1][0]  # last AV emitted: stop

                    for gi, (span, hgaps, placed) in enumerate(groups):
                        sp2 = spp.tile([128, 1536], f32, tag="sp")
                        for kp, off, w, kind, pos in placed:
                            qs = 512 * gq + off
                            nc.tensor.matmul(
                                sp2[:, pos : pos + w],
                                lhsT=kt_sb[:, 128 * kp : 128 * kp + 128],
                                rhs=qt_sb[:, qs : qs + w],
                                start=True,
                                stop=True,
                                skip_group_check=True,
                            )
                        for ga, gb in hgaps:
                            # zero-fill packing holes so the single exp run
                            # never reads stale PSUM
                            nc.tensor.matmul(
                                sp2[:, ga:gb],
                                lhsT=kt_sb[:, 0:128],
                                rhs=zt[:, 0 : gb - ga],
                                start=True,
                                stop=True,
                                skip_group_check=True,
                            )
                        pt = ptp.tile([128, 1536], fp16, tag="pt")
                        nc.scalar.activation(
                            pt[:, 0:span], sp2[:, 0:span], Exp, scale=0.125
                        )
                        for kp, off, w, kind, pos in placed:
                            if kind == 1:
                                nc.vector.tensor_mul(
                                    pt[:, pos : pos + 128],
                                    pt[:, pos : pos + 128],
                                    msk_sb[:, 0:128],
                                )
                            elif kind == 2:
                                nc.vector.tensor_mul(
                                    pt[:, pos : pos + 128],
                                    pt[:, pos : pos + 128],
                                    msk_sb[:, 128:256],
                                )
                            elif kind == 3 and band_w[kp] > 0:
                                blo = int(8 * lo[kp])
                                boff = 256 + int(band_off[kp])
                                nc.vector.tensor_mul(
                                    pt[:, pos + blo : pos + w],
                                    pt[:, pos + blo : pos + w],
                                    msk_sb[:, boff : boff + band_w[kp]],
                                )
                        args = [
                            (kp, pt, pos, off, w, kp == first_kp, kp == last_kp)
                            for kp, off, w, kind, pos in placed
                        ]
                        fin = (h, gq) if gi == len(groups) - 1 else None
                        pending.append((av, vw_sb, args, fin))
                        if len(pending) > PIPE:
                            pop_emit()

            while pending:
                pop_emit()

    import concourse.mybir as mybir_mod

    _split_multi_waits(nc, mybir_mod)
    return nc


def _host_inputs(queries, keys, values, attach, n, lo):
    """Build per-core input maps (host-side layout prep)."""
    f16 = np.float16
    p = np.arange(128)
    f = np.arange(128)
    mdiag = np.where(f[None, :] >= p[:, None], 1.0, 0.0).astype(np.float32)
    mchunk = np.where(
        (p[:, None] // CH == f[None, :] // CH) & (p[:, None] <= f[None, :]),
        1.0,
        0.0,
    ).astype(np.float32)

    band_w = [int(8 * (n[kp] - lo[kp])) for kp in range(NTR)]
    band_off = np.concatenate([[0], np.cumsum(band_w)]).astype(int)
    BAND = int(band_off[-1])

    # per-batch: chunk sort (descending attach) + permuted test row order
    perms = {}
    rowperm = {}
    for b in range(B):
        perm = np.argsort(-attach[b], kind="stable")
        perms[b] = perm
        rowperm[b] = (perm[:, None] * CH + np.arange(CH)[None, :]).reshape(-1)

    in_maps = []
    for c in range(NCORES):
        b, g = divmod(c, 2)
        hs = slice(HPC * g, HPC * (g + 1))
        rp = rowperm[b]
        srt = attach[b][perms[b]]
        q = queries[b][:, hs, :].copy()   # [L, 4, D]
        k = keys[b][:, hs, :].copy()
        v = values[b][:, hs, :].copy()
        q[TRAIN:] = q[TRAIN:][rp]
        k[TRAIN:] = k[TRAIN:][rp]
        v[TRAIN:] = v[TRAIN:][rp]
        kq = np.zeros((HPC, 128, 2 * L), np.float32)
        kq[:, :D, :L] = k.transpose(1, 2, 0)
        kq[:, :D, L:] = q.transpose(1, 2, 0)
        vw = np.empty((HPC, L, 65), np.float32)
        vw[:, :, :64] = v.transpose(1, 0, 2)
        vw[:, :, 64] = 1.0
        # [4, L, 65] -> [4, 128, KT*65] with row p holding tile-chunks
        vw = np.ascontiguousarray(
            vw.reshape(HPC, KT, 128, 65).transpose(0, 2, 1, 3).reshape(HPC, 128, KT * 65)
        )
        msk = np.zeros((128, 256 + BAND), np.float32)
        msk[:, 0:128] = mdiag
        msk[:, 128:256] = mchunk
        for kp in range(NTR):
            if band_w[kp] == 0:
                continue
            ids = np.arange(int(lo[kp]), int(n[kp]))            # chunk slots
            thr = srt[ids]                                      # [w_chunks]
            m = (128 * kp + p[:, None] <= thr[None, :]).astype(np.float32)
            m = np.repeat(m, CH, axis=1)                        # chunk -> 8 cols
            o = 256 + int(band_off[kp])
            msk[:, o : o + band_w[kp]] = m
        in_maps.append(
            {
                "kq": kq.astype(f16),
                "vw": vw.astype(f16),
                "msk": msk.astype(f16),
            }
        )
    return in_maps


def kernel(queries, keys, values, attach_test_after, train_len):
    global LAST_RESULT
    import os

    queries = np.asarray(queries, dtype=np.float32)
    keys = np.asarray(keys, dtype=np.float32)
    values = np.asarray(values, dtype=np.float32)
    attach = np.asarray(attach_test_after).astype(np.int64)
    tl = int(np.asarray(train_len))
    assert queries.shape == (B, L, H, D), queries.shape
    assert tl == TRAIN and attach.shape == (B, NCH)

    from concourse.bass_utils import run_bass_kernel_spmd

    n, lo = _staircase(attach)
    key = (tuple(n.tolist()), tuple(lo.tolist()))
    prog = _PROG_CACHE.get(key)
    if prog is None:
        prog = _build_program(n, lo)
        _PROG_CACHE[key] = prog

    in_maps = _host_inputs(queries, keys, values, attach, n, lo)
    trace = bool(int(os.environ.get("KERNEL_TRACE", "0")))
    res = run_bass_kernel_spmd(
        prog, in_maps, core_ids=list(range(NCORES)), trace=trace
    )
    LAST_RESULT = res

    out = np.empty((B, L, H * D), np.float32)
    inv = {}
    for b in range(B):
        perm = np.argsort(-attach[b], kind="stable")
        rp = (perm[:, None] * CH + np.arange(CH)[None, :]).reshape(-1)
        iv = np.empty(TEST, np.int64)
        iv[rp] = np.arange(TEST)
        inv[b] = iv
    for c in range(NCORES):
        b, g = divmod(c, 2)
        ot = res.results[c]["ot"]                     # [4, 65, L]
        o = ot[:, :64, :] / ot[:, 64:65, :]           # [4, 64, L]
        o = o.transpose(2, 0, 1).reshape(L, HPC * D)
        o[TRAIN:] = o[TRAIN:][inv[b]]
        out[b, :, 256 * g : 256 * (g + 1)] = o
    return out


# revision 24
# speedup vs baseline: 1.0110x; 1.0110x over previous
"""Continual-attention Trainium2 kernel (8 NeuronCores, SPMD).

Sharding: core c -> batch b = c//2, head-group g = c%2 (4 heads each).
Per (b,h) computes S^T[k,q] = K Q^T via PE, exp on ScalarE with fused
1/sqrt(d) scale, 0/1 mask multiplies on DVE, then O^T[d,q] (+ softmax
denominator as a 65th row via a ones column in V) accumulated on PE.

The test-query block (last 512 q) is permuted so its 64 chunks are
sorted by attach_test_after descending; the train-key work for it then
forms a staircase (k-tile kp only needed by the prefix of chunks with
attach >= 128*kp).  The staircase (union over the 4 batches) is baked
into the compiled program -- programs are JIT-specialized per
attach pattern and cached.  Per-core 0/1 boundary masks handle the
sub-tile thresholds (and batches whose own staircase is shorter than
the union).

S tiles are bank-packed first-fit-decreasing into [128,1536] PSUM
tiles (3 banks) so each ScalarE exp covers up to 1536 cols -- ACT is
the bottleneck engine, so fewer/larger exps with no packing holes.
Output is DMA'd straight from PSUM; normalization + transpose on host.
"""

import sys

sys.path.insert(0, "/opt/trn_rl_repo")

import numpy as np

B, L, H, D = 4, 2048, 8, 64
TRAIN = 1536
TEST = L - TRAIN            # 512
NCH = 64                    # test chunks
CH = TEST // NCH            # 8
HPC = 4                     # heads per core
NCORES = 8
KT = L // 128               # 16 k-tiles
NTR = TRAIN // 128          # 12 train k-tiles

LAST_RESULT = None          # BassKernelResults of the most recent run
_PROG_CACHE = {}            # staircase signature -> compiled Bass program


def _split_multi_waits(nc, mybir):
    """This container's walrus accepts at most one semaphore wait per
    instruction; Tile's tail drains can carry several. Hoist extras onto
    NoOps inserted immediately before, on the same engine."""
    for f in nc.m.functions:
        for bb in f.blocks:
            insts = list(bb.instructions)
            out = []
            changed = False
            for inst in insts:
                si = inst.sync_info
                if si is not None and len(si.on_wait) > 1:
                    waits = list(si.on_wait)
                    for w in waits[:-1]:
                        nop = mybir.InstNoOp(
                            name=f"waitnop-{nc.next_id()}", ins=[], outs=[]
                        )
                        nop.engine = inst.engine
                        nop.sync_info = mybir.SyncInfo(on_wait=[w], on_update=[])
                        out.append(nop)
                    inst.sync_info = mybir.SyncInfo(
                        on_wait=[waits[-1]], on_update=list(si.on_update)
                    )
                    changed = True
                out.append(inst)
            if changed:
                bb.instructions = out


def _staircase(attach):
    """From full attach [B, NCH] -> (n, lo) per train k-tile kp:
    n[kp]  = union (max over batches) #chunks needing tile kp (desc-sorted)
    lo[kp] = min over batches of #chunks fully covered by tile kp
    Both in chunk units; the kernel computes cols [0, 8*n) for tile kp and
    mask-multiplies cols [8*lo, 8*n)."""
    srt = -np.sort(-attach, axis=1)          # [B, NCH] descending
    n = np.zeros(NTR, np.int64)
    lo = np.zeros(NTR, np.int64)
    for kp in range(NTR):
        need = (srt >= 128 * kp).sum(axis=1)       # per-batch chunks needing kp
        full = (srt >= 128 * kp + 127).sum(axis=1)  # per-batch fully-allowed
        n[kp] = need.max()
        lo[kp] = full.min()
    return n, lo


def _pack_items(items, first_small=False):
    """items: list of (kp, off, w, kind). First item must stay first
    (its AV carries start=True over the full 512).  Pack into groups of
    <=3 PSUM banks (1536 cols), each item within a single 512-col bank,
    first-fit-decreasing after the first item.  first_small puts the
    first bank alone in group 0 (fast pipeline start).  Returns list of
    groups; each group is (span, gaps, [(kp, off, w, kind, pos), ...])."""
    first, rest = items[0], sorted(items[1:], key=lambda it: -it[2])
    banks = [[first]]  # each bank: list of items, sum w <= 512
    for it in rest:
        for bk in banks:
            if sum(x[2] for x in bk) + it[2] <= 512:
                bk.append(it)
                break
        else:
            banks.append([it])
    # full banks first (stable: kp0's full bank stays first) so partial
    # banks cluster at the end -- gaps between banks stay small
    banks.sort(key=lambda bk: -sum(x[2] for x in bk))
    chunks = []
    if first_small:
        chunks.append(banks[:1])
        banks = banks[1:]
    chunks += [banks[i : i + 3] for i in range(0, len(banks), 3)]
    groups = []
    for chunk in chunks:
        placed = []
        gaps = []  # unwritten [start, end) holes, zero-filled pre-exp
        pos = 0
        for j, bk in enumerate(chunk):
            if j and pos < 512 * j:
                gaps.append((pos, 512 * j))
            pos = 512 * j
            for it in bk:
                placed.append((*it, pos))
                pos += it[2]
        groups.append((pos, gaps, placed))
    return groups


def _gq_items(gq, n):
    """Item list for query-group gq. kind: 0 plain, 1 diag, 2 chunk-block,
    3 staircase (mask cols [8*lo,8*n))."""
    items = []
    if gq < 3:
        for kp in range(4 * (gq + 1)):
            off = max(0, 128 * kp - 512 * gq)
            kind = 1 if 128 * kp >= 512 * gq else 0
            items.append((kp, off, 512 - off, kind))
    else:
        for kp in range(NTR):
            if n[kp] > 0:
                items.append((kp, 0, int(8 * n[kp]), 3))
        for kp in range(NTR, KT):
            items.append((kp, 128 * (kp - NTR), 128, 2))
    return items


def _build_program(n, lo):
    import concourse.bass as bass
    import concourse.mybir as mybir
    import concourse.tile as tile

    f32 = mybir.dt.float32
    fp16 = mybir.dt.float16
    Exp = mybir.ActivationFunctionType.Exp

    band_w = [int(8 * (n[kp] - lo[kp])) for kp in range(NTR)]
    band_off = np.concatenate([[0], np.cumsum(band_w)]).astype(int)
    BAND = int(band_off[-1])

    nc = bass.Bass()

    kq_d = nc.dram_tensor("kq", [HPC, 128, 2 * L], fp16, kind="ExternalInput")
    vw_d = nc.dram_tensor("vw", [HPC, 128, KT * 65], fp16, kind="ExternalInput")
    # masks: [0:128) mdiag | [128:256) mchunk | [256:256+BAND) staircase bands
    msk_d = nc.dram_tensor("msk", [128, 256 + BAND], fp16, kind="ExternalInput")
    ot_d = nc.dram_tensor("ot", [HPC, 65, L], f32, kind="ExternalOutput")

    with tile.TileContext(nc) as tc:
        with (
            tc.tile_pool(name="consts", bufs=1) as consts,
            tc.tile_pool(name="heads", bufs=2) as heads,
            tc.tile_pool(name="ptp", bufs=6) as ptp,
            tc.tile_pool(name="osbp", bufs=3) as osbp,
            tc.tile_pool(name="spp", bufs=2, space="PSUM") as spp,
            tc.tile_pool(name="avp", bufs=2, space="PSUM") as avp,
        ):
            msk_sb = consts.tile([128, 256 + BAND], fp16)
            zt = consts.tile([128, 512], fp16)
            nc.vector.memset(zt, 0.0)
            # dummy activation: hoists the Exp table load to t~0, off the
            # critical path (otherwise it serializes before the first exp)
            warm = consts.tile([1, 8], f32)
            nc.scalar.activation(warm, zt[0:1, 0:8], Exp)

            # AV matmuls are held back PIPE groups and flow across gq/head
            # boundaries so the PE never serializes a drain in front of the
            # next group's S matmuls (which would starve ScalarE).
            PIPE = 2
            pending = []  # entries: (av, vw_sb, [(kp,pt,pos,off,w,start,stop)], fin)

            def pop_emit():
                av_e, vw_e, args, fin = pending.pop(0)
                for kp, pt, pos, off, w, st, sp in args:
                    nc.tensor.matmul(
                        av_e[:65, off : off + w],
                        lhsT=vw_e[:, kp, :],
                        rhs=pt[:, pos : pos + w],
                        start=st,
                        stop=sp,
                        skip_group_check=True,
                    )
                if fin is not None:
                    h_e, gq_e = fin
                    od = ot_d.ap()[h_e][:, 512 * gq_e : 512 * gq_e + 512]
                    if (h_e, gq_e) == (HPC - 1, 3):
                        # final output: copy+DMA in halves so the last DMA
                        # overlaps the second copy (shorter drain tail)
                        for half in range(2):
                            sl = slice(256 * half, 256 * half + 256)
                            osb = osbp.tile([65, 256], f32, tag="osbh")
                            nc.vector.tensor_copy(osb, av_e[:65, sl])
                            nc.sync.dma_start(out=od[:, sl], in_=osb)
                    else:
                        osb = osbp.tile([65, 512], f32)
                        nc.vector.tensor_copy(osb, av_e[:65, :])
                        nc.sync.dma_start(out=od, in_=osb)

            first = True
            for h in range(HPC):
                # qt/kt are zero-padded to 128 contraction rows: K=64 matmuls
                # never leave the PE's throttled clock state (HW-measured
                # 430ns vs 216ns per 512-col matmul), K=128 ones do.
                # head-0 DMAs are ordered/split so compute starts as soon as
                # the first k/q columns land; qt goes via the ACT-engine DGE
                # so the sync sequencer isn't the serial bottleneck.
                kq_sb = heads.tile([128, 2 * L], fp16, tag="kq")
                kt_sb = kq_sb[:, 0:L]
                qt_sb = kq_sb[:, L : 2 * L]
                vw_sb = heads.tile([128, KT, 65], fp16, tag="vw")
                kq2 = kq_sb.rearrange("p (b c) -> p b c", b=2)
                kqd2 = kq_d.ap()[h].rearrange("p (b c) -> p b c", b=2)
                if first:
                    # first chunk: both k and q cols [0:512), real rows only
                    # (fewest descriptors + one issue/sem chain), zero rows
                    # via idle gpsimd; bulk bytes issued after so they don't
                    # queue in front on the DMA engines
                    nc.gpsimd.memset(kt_sb[64:128, 0:512], 0.0)
                    nc.gpsimd.memset(qt_sb[64:128, 0:512], 0.0)
                    nc.sync.dma_start(
                        out=kq2[0:64, :, 0:512], in_=kqd2[0:64, :, 0:512]
                    )
                    nc.sync.dma_start(out=msk_sb, in_=msk_d.ap())
                    nc.scalar.dma_start(
                        out=kq2[:, :, 512:L], in_=kqd2[:, :, 512:L]
                    )
                    vw_r = vw_d.ap()[h].rearrange("p (t c) -> p t c", t=KT)
                    nc.sync.dma_start(out=vw_sb[:, 0:4, :], in_=vw_r[:, 0:4, :])
                    nc.sync.dma_start(out=vw_sb[:, 4:KT, :], in_=vw_r[:, 4:KT, :])
                    first = False
                else:
                    # on the sync DGE: a scalar-engine dma_start mid-stream
                    # would stall the exp queue for its ~630ns HWDGE issue
                    nc.sync.dma_start(out=kq_sb, in_=kq_d.ap()[h])
                    nc.sync.dma_start(
                        out=vw_sb,
                        in_=vw_d.ap()[h].rearrange("p (t c) -> p t c", t=KT),
                    )

                for gq in range(4):
                    av = avp.tile([128, 512], f32, tag="av")
                    items = _gq_items(gq, n)
                    groups = _pack_items(items, first_small=(h == 0 and gq == 0))
                    first_kp = items[0][0]          # first AV emitted: start
                    last_kp = groups[-1][2][-1][0]  # last AV emitted: stop
                    # near the head boundary let the AV backlog grow so the
                    # next head's first S groups aren't queued behind it on
                    # the PE (which would starve ScalarE at the transition)
                    defer = 2 if (gq == 3 and h < HPC - 1) else 0

                    for gi, (span, hgaps, placed) in enumerate(groups):
                        sp2 = spp.tile([128, 1536], f32, tag="sp")
                        for kp, off, w, kind, pos in placed:
                            qs = 512 * gq + off
                            nc.tensor.matmul(
                                sp2[:, pos : pos + w],
                                lhsT=kt_sb[:, 128 * kp : 128 * kp + 128],
                                rhs=qt_sb[:, qs : qs + w],
                                start=True,
                                stop=True,
                                skip_group_check=True,
                            )
                        for ga, gb in hgaps:
                            # zero-fill packing holes so the single exp run
                            # never reads stale PSUM
                            nc.tensor.matmul(
                                sp2[:, ga:gb],
                                lhsT=kt_sb[:, 0:128],
                                rhs=zt[:, 0 : gb - ga],
                                start=True,
                                stop=True,
                                skip_group_check=True,
                            )
                        pt = ptp.tile([128, 1536], fp16, tag="pt")
                        nc.scalar.activation(
                            pt[:, 0:span], sp2[:, 0:span], Exp, scale=0.125
                        )
                        for kp, off, w, kind, pos in placed:
                            if kind == 1:
                                nc.vector.tensor_mul(
                                    pt[:, pos : pos + 128],
                                    pt[:, pos : pos + 128],
                                    msk_sb[:, 0:128],
                                )
                            elif kind == 2:
                                nc.vector.tensor_mul(
                                    pt[:, pos : pos + 128],
                                    pt[:, pos : pos + 128],
                                    msk_sb[:, 128:256],
                                )
                            elif kind == 3 and band_w[kp] > 0:
                                blo = int(8 * lo[kp])
                                boff = 256 + int(band_off[kp])
                                nc.vector.tensor_mul(
                                    pt[:, pos + blo : pos + w],
                                    pt[:, pos + blo : pos + w],
                                    msk_sb[:, boff : boff + band_w[kp]],
                                )
                        args = [
                            (kp, pt, pos, off, w, kp == first_kp, kp == last_kp)
                            for kp, off, w, kind, pos in placed
                        ]
                        fin = (h, gq) if gi == len(groups) - 1 else None
                        pending.append((av, vw_sb, args, fin))
                        hold = PIPE + (defer if gi >= len(groups) - 2 else 0)
                        # drain at most 2 per unit so a deferred backlog
                        # doesn't stall the PE all at once
                        pops = min(max(len(pending) - hold, 0), 2)
                        for _ in range(pops):
                            pop_emit()

            while pending:
                pop_emit()

    import concourse.mybir as mybir_mod

    _split_multi_waits(nc, mybir_mod)
    return nc


def _host_inputs(queries, keys, values, attach, n, lo):
    """Build per-core input maps (host-side layout prep)."""
    f16 = np.float16
    p = np.arange(128)
    f = np.arange(128)
    mdiag = np.where(f[None, :] >= p[:, None], 1.0, 0.0).astype(np.float32)
    mchunk = np.where(
        (p[:, None] // CH == f[None, :] // CH) & (p[:, None] <= f[None, :]),
        1.0,
        0.0,
    ).astype(np.float32)

    band_w = [int(8 * (n[kp] - lo[kp])) for kp in range(NTR)]
    band_off = np.concatenate([[0], np.cumsum(band_w)]).astype(int)
    BAND = int(band_off[-1])

    # per-batch: chunk sort (descending attach) + permuted test row order
    perms = {}
    rowperm = {}
    for b in range(B):
        perm = np.argsort(-attach[b], kind="stable")
        perms[b] = perm
        rowperm[b] = (perm[:, None] * CH + np.arange(CH)[None, :]).reshape(-1)

    in_maps = []
    for c in range(NCORES):
        b, g = divmod(c, 2)
        hs = slice(HPC * g, HPC * (g + 1))
        rp = rowperm[b]
        srt = attach[b][perms[b]]
        q = queries[b][:, hs, :].copy()   # [L, 4, D]
        k = keys[b][:, hs, :].copy()
        v = values[b][:, hs, :].copy()
        q[TRAIN:] = q[TRAIN:][rp]
        k[TRAIN:] = k[TRAIN:][rp]
        v[TRAIN:] = v[TRAIN:][rp]
        kq = np.zeros((HPC, 128, 2 * L), np.float32)
        kq[:, :D, :L] = k.transpose(1, 2, 0)
        kq[:, :D, L:] = q.transpose(1, 2, 0)
        vw = np.empty((HPC, L, 65), np.float32)
        vw[:, :, :64] = v.transpose(1, 0, 2)
        vw[:, :, 64] = 1.0
        # [4, L, 65] -> [4, 128, KT*65] with row p holding tile-chunks
        vw = np.ascontiguousarray(
            vw.reshape(HPC, KT, 128, 65).transpose(0, 2, 1, 3).reshape(HPC, 128, KT * 65)
        )
        msk = np.zeros((128, 256 + BAND), np.float32)
        msk[:, 0:128] = mdiag
        msk[:, 128:256] = mchunk
        for kp in range(NTR):
            if band_w[kp] == 0:
                continue
            ids = np.arange(int(lo[kp]), int(n[kp]))            # chunk slots
            thr = srt[ids]                                      # [w_chunks]
            m = (128 * kp + p[:, None] <= thr[None, :]).astype(np.float32)
            m = np.repeat(m, CH, axis=1)                        # chunk -> 8 cols
            o = 256 + int(band_off[kp])
            msk[:, o : o + band_w[kp]] = m
        in_maps.append(
            {
                "kq": kq.astype(f16),
                "vw": vw.astype(f16),
                "msk": msk.astype(f16),
            }
        )
    return in_maps


def kernel(queries, keys, values, attach_test_after, train_len):
    global LAST_RESULT
    import os

    queries = np.asarray(queries, dtype=np.float32)
    keys = np.asarray(keys, dtype=np.float32)
    values = np.asarray(values, dtype=np.float32)
    attach = np.asarray(attach_test_after).astype(np.int64)
    tl = int(np.asarray(train_len))
    assert queries.shape == (B, L, H, D), queries.shape
    assert tl == TRAIN and attach.shape == (B, NCH)

    from concourse.bass_utils import run_bass_kernel_spmd

    n, lo = _staircase(attach)
    key = (tuple(n.tolist()), tuple(lo.tolist()))
    prog = _PROG_CACHE.get(key)
    if prog is None:
        prog = _build_program(n, lo)
        _PROG_CACHE[key] = prog

    in_maps = _host_inputs(queries, keys, values, attach, n, lo)
    trace = bool(int(os.environ.get("KERNEL_TRACE", "0")))
    res = run_bass_kernel_spmd(
        prog, in_maps, core_ids=list(range(NCORES)), trace=trace
    )
    LAST_RESULT = res

    out = np.empty((B, L, H * D), np.float32)
    inv = {}
    for b in range(B):
        perm = np.argsort(-attach[b], kind="stable")
        rp = (perm[:, None] * CH + np.arange(CH)[None, :]).reshape(-1)
        iv = np.empty(TEST, np.int64)
        iv[rp] = np.arange(TEST)
        inv[b] = iv
    for c in range(NCORES):
        b, g = divmod(c, 2)
        ot = res.results[c]["ot"]                     # [4, 65, L]
        o = ot[:, :64, :] / ot[:, 64:65, :]           # [4, 64, L]
        o = o.transpose(2, 0, 1).reshape(L, HPC * D)
        o[TRAIN:] = o[TRAIN:][inv[b]]
        out[b, :, 256 * g : 256 * (g + 1)] = o
    return out
